# revision 3
# baseline (speedup 1.0000x reference)
"""Trainium2 Bass kernel for causal multi-head attention with QKV projections.

v3 (on top of v2a's col-tiled AV + ES denominators + O^T output):
  - Stages split into QK-projection and V parts; the first exp fires ~15us
    earlier since it only gates on q/k chunk 0.
  - V^T -> V transposes moved off the PE onto the DMA xbar
    (dma_start_transpose, one call per (pair, 512-chunk) with a 3D out AP).
  - Diagonal-chunk tiles are column-trimmed: QK n, exp N, ES add and AV n
    only cover the causally-valid query window; the causal triangle mask
    shrinks to one 128-col block per head.
"""

import sys

import numpy as np

try:
    import concourse  # noqa: F401
except ImportError:  # pragma: no cover
    sys.path.insert(0, "/opt/trn_rl_repo")

from contextlib import ExitStack

import ml_dtypes
import concourse.bass as bass  # noqa: F401
import concourse.tile as tile
from concourse import bacc, bass_utils, mybir

FP = mybir.dt.float32
FR = mybir.dt.float32r
BF = mybir.dt.bfloat16
AF = mybir.ActivationFunctionType

B, T_FULL, C = 2, 4096, 1024
H, D = 16, 64
N_CORES = 8
HPC = 4            # heads per core
CPC = HPC * D      # channels per core (256)
QG = 512           # query-group width

_CACHE = {}


def _emit(ctx, tc, t):
    nc = tc.nc
    nkt = t // 128       # key tiles
    nqg = t // QG        # query groups
    ntc = t // 512       # projection t-chunks

    xT = nc.dram_tensor("xT", [C, t], BF, kind="ExternalInput").ap()
    wqT = nc.dram_tensor("wqT", [C, CPC], BF, kind="ExternalInput").ap()
    wkT = nc.dram_tensor("wkT", [C, CPC], BF, kind="ExternalInput").ap()
    wvT = nc.dram_tensor("wvT", [C, CPC], BF, kind="ExternalInput").ap()
    out = nc.dram_tensor("out", [CPC, t], FP, kind="ExternalOutput").ap()

    # ---------------- persistent SBUF ----------------
    big = ctx.enter_context(tc.tile_pool(name="big", bufs=1))
    ones_bf = big.tile([128, 64], BF, tag="ones_bf")
    nc.vector.memset(ones_bf, 1.0)

    qt = [big.tile([128, t], BF, tag=f"qt{hp}", name=f"qt{hp}") for hp in range(HPC // 2)]
    kt = [big.tile([128, t], BF, tag=f"kt{hp}", name=f"kt{hp}") for hp in range(HPC // 2)]
    # V for both heads of a pair: [keys(128), pair, ktile, dims(128: hA 0-63, hB 64-127)]
    v_sb = big.tile([128, HPC // 2, nkt, 128], BF, tag="v_sb")
    # V^T staging (proj output, [2 heads x 64 dims, keys]) per pair, bf16 for xbar
    vt = [big.tile([128, t], BF, tag=f"vt{hp}", name=f"vt{hp}") for hp in range(HPC // 2)]

    xT_r = xT.rearrange("(k p) t -> p k t", p=128)
    pn = 512

    # x stays resident: both head-pairs reuse it (xT read once, not twice).
    x_all = big.tile([128, ntc, 8, pn], BF, tag="x_all")
    # chunk 0 k-split so the first projection matmul is fed early
    for kk in range(4):
        nc.sync.dma_start(
            x_all[:, 0, 2 * kk : 2 * kk + 2, :], xT_r[:, 2 * kk : 2 * kk + 2, 0:pn]
        )
    w_sb = {}
    for name, dram in (("wq", wqT), ("wk", wkT), ("wv", wvT)):
        w_sb[name] = big.tile([128, 8, CPC], BF, tag=name, name=f"w_{name}")
        nc.sync.dma_start(w_sb[name], dram.rearrange("(k p) m -> p k m", p=128))

    spsum = ctx.enter_context(tc.tile_pool(name="spsum", bufs=2, space="PSUM"))
    opsum = ctx.enter_context(tc.tile_pool(name="opsum", bufs=2, space="PSUM"))
    fillp = ctx.enter_context(tc.tile_pool(name="fillp", bufs=2, space="PSUM"))
    ep = ctx.enter_context(tc.tile_pool(name="ep", bufs=10))
    fin = ctx.enter_context(tc.tile_pool(name="fin", bufs=2))
    esp = ctx.enter_context(tc.tile_pool(name="esp", bufs=2))

    # PE warm-up: ~9 dummy matmuls while the first DMAs land, so the HAM
    # clock gate reaches 8/8 before the first projection matmuls run.
    warm = big.tile([128, 512], BF, tag="warm")
    nc.vector.memset(warm, 0.0)
    for wi in range(9):
        wp = fillp.tile([128, 512], FP, tag="fill", name=f"warm{wi}")
        nc.tensor.matmul(wp[0:64, :], lhsT=ones_bf, rhs=warm, start=True, stop=True)

    # ---- staged filler: qk-projections | v-projection + xbar transpose ----
    def make_stage_qk(hp, ch):
        def gen():
            tsl = slice(ch * pn, (ch + 1) * pn)
            if hp == 0 and ch > 0:
                nc.sync.dma_start(x_all[:, ch], xT_r[:, :, tsl])
                yield
            for w_tile, dst in ((w_sb["wq"], qt[hp]), (w_sb["wk"], kt[hp])):
                pp = fillp.tile([128, pn], FP, tag="fill", name=f"ppqk{hp}_{ch}")
                for k in range(8):
                    nc.tensor.matmul(
                        pp,
                        lhsT=w_tile[:, k, hp * 128 : (hp + 1) * 128],
                        rhs=x_all[:, ch, k, :],
                        start=(k == 0),
                        stop=(k == 7),
                    )
                    yield
                nc.vector.tensor_copy(dst[:, tsl], pp)
                yield
        return gen()

    def make_stage_v(hp, ch):
        def gen():
            tsl = slice(ch * pn, (ch + 1) * pn)
            pp = fillp.tile([128, pn], FP, tag="fill", name=f"ppv{hp}_{ch}")
            for k in range(8):
                nc.tensor.matmul(
                    pp,
                    lhsT=w_sb["wv"][:, k, hp * 128 : (hp + 1) * 128],
                    rhs=x_all[:, ch, k, :],
                    start=(k == 0),
                    stop=(k == 7),
                )
                yield
            nc.vector.tensor_copy(vt[hp][:, tsl], pp)
            yield
            # xbar transpose: [128 (2h x 64d), 512 keys] -> [512 keys, 128]
            # blocked into v_sb[:, hp, 4ch:4ch+4, :] (3D out AP)
            nc.sync.dma_start_transpose(
                v_sb[:, hp, 4 * ch : 4 * ch + 4, :], vt[hp][:, tsl]
            )
            yield
        return gen()

    stages = []
    stage_yields = []
    for hp in range(HPC // 2):
        for ch in range(ntc):
            stages.append(make_stage_qk(hp, ch))
            stage_yields.append(19 if (hp == 0 and ch > 0) else 18)
            stages.append(make_stage_v(hp, ch))
            stage_yields.append(10)
    cum_yields = []
    tot = 0
    for y in stage_yields:
        tot += y
        cum_yields.append(tot)
    cursor = {"i": 0, "done": 0}

    def feed(n):
        done = 0
        while done < n and cursor["i"] < len(stages):
            if next(stages[cursor["i"]], StopIteration) is StopIteration:
                cursor["i"] += 1
            else:
                done += 1
                cursor["done"] += 1

    def gate(s_idx):
        while cursor["i"] <= s_idx:
            if next(stages[cursor["i"]], StopIteration) is StopIteration:
                cursor["i"] += 1
            else:
                cursor["done"] += 1

    def feed_paced(deadline_idx, j_left):
        """Spread the yields needed to finish stages <= deadline_idx over
        the remaining j iterations of the current group."""
        deadline_idx = min(deadline_idx, len(stages) - 1)
        needed = cum_yields[deadline_idx] - cursor["done"]
        if needed > 0:
            feed(-(-needed // max(j_left, 1)))

    # ---------------- attention ----------------
    def run_group(hp, g):
        qsl = slice(g * QG, (g + 1) * QG)
        njs = (g + 1) * (QG // 128)
        st = {"ot": None, "es": None}

        def emit_qk(j):
            m = j - (g * QG) // 128
            w0 = max(m, 0) * 128          # first causally-valid query col
            sp = spsum.tile([128, 2, QG], FP, tag="sp", name=f"sp{hp}_{g}_{j}")
            e = ep.tile([128, 2, QG], BF, tag="e", name=f"e{hp}_{g}_{j}")
            for hh in (0, 1):
                po = 64 * hh
                nc.tensor.matmul(
                    sp[:, hh, w0:],
                    lhsT=kt[hp][po : po + 64, j * 128 : (j + 1) * 128],
                    rhs=qt[hp][po : po + 64, g * QG + w0 : (g + 1) * QG],
                    start=True,
                    stop=True,
                )
            nc.scalar.activation(
                out=e[:, :, w0:],
                in_=sp[:, :, w0:],
                func=AF.Exp,
            )
            if m >= 0:  # causal triangle on the diagonal 128-col block
                for hh in (0, 1):
                    blk = e[:, hh, w0 : w0 + 128]
                    nc.gpsimd.affine_select(
                        out=blk,
                        in_=blk,
                        compare_op=mybir.AluOpType.is_ge,
                        fill=0.0,
                        base=0,
                        pattern=[[1, 128]],
                        channel_multiplier=-1,
                    )
            # running E sum (denominator source)
            if st["es"] is None:
                st["es"] = esp.tile([128, 2, QG], BF, tag="es", name=f"es{hp}_{g}")
                nc.vector.tensor_copy(st["es"], e)  # j==0 is always full-width
            else:
                nc.vector.tensor_add(
                    st["es"][:, :, w0:], st["es"][:, :, w0:], e[:, :, w0:]
                )
            return e, w0

        def emit_av(j, e, w0):
            if st["ot"] is None:
                st["ot"] = opsum.tile([128, QG], FP, tag="ot", name=f"ot{hp}_{g}")
            for hh in (0, 1):
                nc.tensor.matmul(
                    st["ot"][64 * hh : 64 * (hh + 1), w0:],
                    lhsT=v_sb[:, hp, j, 64 * hh : 64 * (hh + 1)],
                    rhs=e[:, hh, w0:],
                    start=(j == 0),
                    stop=(j == njs - 1),
                )
            if j == njs - 1:
                finalize()

        def finalize():
            # denominator broadcast: dbc[p, q] = sum_k ES[k, head(p), q]
            dbc = fillp.tile([128, QG], FP, tag="fill", name=f"dbc{hp}_{g}")
            for hh in (0, 1):
                nc.tensor.matmul(
                    dbc[64 * hh : 64 * (hh + 1), :],
                    lhsT=ones_bf,
                    rhs=st["es"][:, hh, :],
                    start=True,
                    stop=True,
                )
            recip = fin.tile([128, QG], FP, tag="recip", name=f"rc{hp}_{g}")
            nc.vector.reciprocal_approx_fast(recip, dbc)
            o_sb = fin.tile([128, QG], FP, tag="o_sb", name=f"o{hp}_{g}")
            nc.vector.tensor_mul(o_sb, st["ot"], recip)
            nc.sync.dma_start(out[hp * 128 : (hp + 1) * 128, qsl], o_sb)

        return emit_qk, emit_av, njs

    pending = []
    for hp in range(HPC // 2):
        for g in range(nqg):
            gate(2 * (hp * ntc + g))      # q/k of chunks <= g ready
            emit_qk, emit_av, njs = run_group(hp, g)
            gated_v = False
            # pace: next group's qk+v stages should finish by group end
            deadline = 2 * (hp * ntc + g + 1) + 1
            for j in range(njs):
                e, w0 = emit_qk(j)
                pending.append((emit_av, j, e, w0))
                feed_paced(deadline, 2 * (njs - j))
                if not gated_v:
                    gate(2 * (hp * ntc + g) + 1)  # v of chunks <= g ready
                    gated_v = True
                if len(pending) > 5:
                    av, jj, e, ww = pending.pop(0)
                    av(jj, e, ww)
                    feed_paced(deadline, 2 * (njs - j) - 1)
    for av, jj, e, ww in pending:
        av(jj, e, ww)
    feed(10 ** 9)


def build_program(t=T_FULL):
    if t in _CACHE:
        return _CACHE[t]
    nc = bacc.Bacc("TRN2", target_bir_lowering=False, debug=False)
    with tile.TileContext(nc) as tc:
        with ExitStack() as ctx:
            _emit(ctx, tc, t)
    nc.compile()
    _CACHE[t] = nc
    return nc


def make_in_maps(x, Wq, Wk, Wv):
    """Host-side shard: returns the 8 per-core input maps."""
    x = np.asarray(x, dtype=np.float32)
    Wq = np.asarray(Wq, dtype=np.float32)
    Wk = np.asarray(Wk, dtype=np.float32)
    Wv = np.asarray(Wv, dtype=np.float32)
    scale = np.float32(D ** -0.5)
    bf = ml_dtypes.bfloat16
    xT = np.ascontiguousarray(x.transpose(0, 2, 1)).astype(bf)  # [B, C, T]
    in_maps = []
    for core in range(N_CORES):
        b, hg = divmod(core, N_CORES // B)
        sl = slice(hg * CPC, (hg + 1) * CPC)
        in_maps.append(
            {
                "xT": xT[b],
                "wqT": (np.ascontiguousarray(Wq[sl].T) * scale).astype(bf),
                "wkT": np.ascontiguousarray(Wk[sl].T).astype(bf),
                "wvT": np.ascontiguousarray(Wv[sl].T).astype(bf),
            }
        )
    return in_maps


LAST_RESULTS = None


def kernel(x, Wq, Wk, Wv, _trace=False):
    global LAST_RESULTS
    in_maps = make_in_maps(x, Wq, Wk, Wv)
    nc = build_program(T_FULL)
    res = bass_utils.run_bass_kernel_spmd(
        nc, in_maps, core_ids=list(range(N_CORES)), trace=_trace
    )
    LAST_RESULTS = res
    full = np.empty((B, T_FULL, C), np.float32)
    for core in range(N_CORES):
        b, hg = divmod(core, N_CORES // B)
        full[b, :, hg * CPC : (hg + 1) * CPC] = res.results[core]["out"].T
    return full


# revision 5
# speedup vs baseline: 1.0432x; 1.0432x over previous
"""Trainium2 Bass kernel for causal multi-head attention with QKV projections.

v3 (on top of v2a's col-tiled AV + ES denominators + O^T output):
  - Stages split into QK-projection and V parts; the first exp fires ~15us
    earlier since it only gates on q/k chunk 0.
  - V^T -> V transposes moved off the PE onto the DMA xbar
    (dma_start_transpose, one call per (pair, 512-chunk) with a 3D out AP).
  - Diagonal-chunk tiles are column-trimmed: QK n, exp N, ES add and AV n
    only cover the causally-valid query window; the causal triangle mask
    shrinks to one 128-col block per head.
"""

import sys

import numpy as np

try:
    import concourse  # noqa: F401
except ImportError:  # pragma: no cover
    sys.path.insert(0, "/opt/trn_rl_repo")

from contextlib import ExitStack

import ml_dtypes
import concourse.bass as bass  # noqa: F401
import concourse.tile as tile
from concourse import bacc, bass_utils, mybir

FP = mybir.dt.float32
FR = mybir.dt.float32r
BF = mybir.dt.bfloat16
AF = mybir.ActivationFunctionType

B, T_FULL, C = 2, 4096, 1024
# Schraudolph exp on DVE for every OFF-th score tile (rebalances ACT -> DVE):
# bf16 bits = round(s * 128/ln2 + (127*128 - 5.5)), ~3.3% max elementwise
# error, which cancels between softmax numerator and denominator.
EXP_OFF = 8
SCHR_A = 128.0 / float(np.log(2.0))
SCHR_B = 127.0 * 128.0 - 5.5
H, D = 16, 64
N_CORES = 8
HPC = 4            # heads per core
CPC = HPC * D      # channels per core (256)
QG = 512           # query-group width

_CACHE = {}


def _emit(ctx, tc, t):
    nc = tc.nc
    nkt = t // 128       # key tiles
    nqg = t // QG        # query groups
    ntc = t // 512       # projection t-chunks

    xT = nc.dram_tensor("xT", [C, t], BF, kind="ExternalInput").ap()
    wqT = nc.dram_tensor("wqT", [C, CPC], BF, kind="ExternalInput").ap()
    wkT = nc.dram_tensor("wkT", [C, CPC], BF, kind="ExternalInput").ap()
    wvT = nc.dram_tensor("wvT", [C, CPC], BF, kind="ExternalInput").ap()
    out = nc.dram_tensor("out", [CPC, t], FP, kind="ExternalOutput").ap()

    # ---------------- persistent SBUF ----------------
    big = ctx.enter_context(tc.tile_pool(name="big", bufs=1))
    ones_bf = big.tile([128, 64], BF, tag="ones_bf")
    nc.vector.memset(ones_bf, 1.0)

    qt = [big.tile([128, t], BF, tag=f"qt{hp}", name=f"qt{hp}") for hp in range(HPC // 2)]
    kt = [big.tile([128, t], BF, tag=f"kt{hp}", name=f"kt{hp}") for hp in range(HPC // 2)]
    # V for both heads of a pair: [keys(128), pair, ktile, dims(128: hA 0-63, hB 64-127)]
    v_sb = big.tile([128, HPC // 2, nkt, 128], BF, tag="v_sb")
    # V^T staging (proj output, [2 heads x 64 dims, keys]) per pair, bf16 for xbar
    vt = [big.tile([128, t], BF, tag=f"vt{hp}", name=f"vt{hp}") for hp in range(HPC // 2)]

    xT_r = xT.rearrange("(k p) t -> p k t", p=128)
    pn = 512

    # x stays resident: both head-pairs reuse it (xT read once, not twice).
    x_all = big.tile([128, ntc, 8, pn], BF, tag="x_all")
    # chunk 0 k-split so the first projection matmul is fed early
    for kk in range(4):
        nc.sync.dma_start(
            x_all[:, 0, 2 * kk : 2 * kk + 2, :], xT_r[:, 2 * kk : 2 * kk + 2, 0:pn]
        )
    w_sb = {}
    for name, dram in (("wq", wqT), ("wk", wkT), ("wv", wvT)):
        w_sb[name] = big.tile([128, 8, CPC], BF, tag=name, name=f"w_{name}")
        nc.sync.dma_start(w_sb[name], dram.rearrange("(k p) m -> p k m", p=128))

    spsum = ctx.enter_context(tc.tile_pool(name="spsum", bufs=2, space="PSUM"))
    opsum = ctx.enter_context(tc.tile_pool(name="opsum", bufs=2, space="PSUM"))
    fillp = ctx.enter_context(tc.tile_pool(name="fillp", bufs=2, space="PSUM"))
    ep = ctx.enter_context(tc.tile_pool(name="ep", bufs=10))
    fin = ctx.enter_context(tc.tile_pool(name="fin", bufs=2))
    esp = ctx.enter_context(tc.tile_pool(name="esp", bufs=2))

    # PE warm-up: ~9 dummy matmuls while the first DMAs land, so the HAM
    # clock gate reaches 8/8 before the first projection matmuls run.
    warm = big.tile([128, 512], BF, tag="warm")
    nc.vector.memset(warm, 0.0)
    for wi in range(9):
        wp = fillp.tile([128, 512], FP, tag="fill", name=f"warm{wi}")
        nc.tensor.matmul(wp[0:64, :], lhsT=ones_bf, rhs=warm, start=True, stop=True)

    # ---- staged filler: qk-projections | v-projection + xbar transpose ----
    def make_stage_qk(hp, ch):
        def gen():
            tsl = slice(ch * pn, (ch + 1) * pn)
            if hp == 0 and ch > 0:
                nc.sync.dma_start(x_all[:, ch], xT_r[:, :, tsl])
                yield
            for w_tile, dst in ((w_sb["wq"], qt[hp]), (w_sb["wk"], kt[hp])):
                pp = fillp.tile([128, pn], FP, tag="fill", name=f"ppqk{hp}_{ch}")
                for k in range(8):
                    nc.tensor.matmul(
                        pp,
                        lhsT=w_tile[:, k, hp * 128 : (hp + 1) * 128],
                        rhs=x_all[:, ch, k, :],
                        start=(k == 0),
                        stop=(k == 7),
                    )
                    yield
                nc.vector.tensor_copy(dst[:, tsl], pp)
                yield
        return gen()

    def make_stage_v(hp, ch):
        def gen():
            tsl = slice(ch * pn, (ch + 1) * pn)
            pp = fillp.tile([128, pn], FP, tag="fill", name=f"ppv{hp}_{ch}")
            for k in range(8):
                nc.tensor.matmul(
                    pp,
                    lhsT=w_sb["wv"][:, k, hp * 128 : (hp + 1) * 128],
                    rhs=x_all[:, ch, k, :],
                    start=(k == 0),
                    stop=(k == 7),
                )
                yield
            nc.vector.tensor_copy(vt[hp][:, tsl], pp)
            yield
            # xbar transpose: [128 (2h x 64d), 512 keys] -> [512 keys, 128]
            # blocked into v_sb[:, hp, 4ch:4ch+4, :] (3D out AP)
            nc.sync.dma_start_transpose(
                v_sb[:, hp, 4 * ch : 4 * ch + 4, :], vt[hp][:, tsl]
            )
            yield
        return gen()

    stages = []
    stage_yields = []
    for hp in range(HPC // 2):
        for ch in range(ntc):
            stages.append(make_stage_qk(hp, ch))
            stage_yields.append(19 if (hp == 0 and ch > 0) else 18)
            stages.append(make_stage_v(hp, ch))
            stage_yields.append(10)
    cum_yields = []
    tot = 0
    for y in stage_yields:
        tot += y
        cum_yields.append(tot)
    cursor = {"i": 0, "done": 0}

    def feed(n):
        done = 0
        while done < n and cursor["i"] < len(stages):
            if next(stages[cursor["i"]], StopIteration) is StopIteration:
                cursor["i"] += 1
            else:
                done += 1
                cursor["done"] += 1

    def gate(s_idx):
        while cursor["i"] <= s_idx:
            if next(stages[cursor["i"]], StopIteration) is StopIteration:
                cursor["i"] += 1
            else:
                cursor["done"] += 1

    def feed_paced(deadline_idx, j_left):
        """Spread the yields needed to finish stages <= deadline_idx over
        the remaining j iterations of the current group."""
        deadline_idx = min(deadline_idx, len(stages) - 1)
        needed = cum_yields[deadline_idx] - cursor["done"]
        if needed > 0:
            feed(-(-needed // max(j_left, 1)))

    # ---------------- attention ----------------
    tile_ctr = {"n": 0}

    def run_group(hp, g):
        qsl = slice(g * QG, (g + 1) * QG)
        njs = (g + 1) * (QG // 128)
        st = {"ot": None, "es": None}

        def emit_qk(j):
            m = j - (g * QG) // 128
            w0 = max(m, 0) * 128          # first causally-valid query col
            sp = spsum.tile([128, 2, QG], FP, tag="sp", name=f"sp{hp}_{g}_{j}")
            e = ep.tile([128, 2, QG], BF, tag="e", name=f"e{hp}_{g}_{j}")
            for hh in (0, 1):
                po = 64 * hh
                nc.tensor.matmul(
                    sp[:, hh, w0:],
                    lhsT=kt[hp][po : po + 64, j * 128 : (j + 1) * 128],
                    rhs=qt[hp][po : po + 64, g * QG + w0 : (g + 1) * QG],
                    start=True,
                    stop=True,
                )
            tile_ctr["n"] += 1
            if tile_ctr["n"] % EXP_OFF == 0:
                # Schraudolph approx exp on DVE (offloads the ACT chain)
                nc.vector.tensor_scalar(
                    out=e[:, :, w0:].bitcast(mybir.dt.int16),
                    in0=sp[:, :, w0:],
                    scalar1=SCHR_A,
                    scalar2=SCHR_B,
                    op0=mybir.AluOpType.mult,
                    op1=mybir.AluOpType.add,
                )
            else:
                nc.scalar.activation(
                    out=e[:, :, w0:],
                    in_=sp[:, :, w0:],
                    func=AF.Exp,
                )
            if m >= 0:  # causal triangle on the diagonal 128-col block
                for hh in (0, 1):
                    blk = e[:, hh, w0 : w0 + 128]
                    nc.gpsimd.affine_select(
                        out=blk,
                        in_=blk,
                        compare_op=mybir.AluOpType.is_ge,
                        fill=0.0,
                        base=0,
                        pattern=[[1, 128]],
                        channel_multiplier=-1,
                    )
            # running E sum (denominator source)
            if st["es"] is None:
                st["es"] = esp.tile([128, 2, QG], BF, tag="es", name=f"es{hp}_{g}")
                nc.vector.tensor_copy(st["es"], e)  # j==0 is always full-width
            else:
                nc.vector.tensor_add(
                    st["es"][:, :, w0:], st["es"][:, :, w0:], e[:, :, w0:]
                )
            return e, w0

        def emit_av(j, e, w0):
            if st["ot"] is None:
                st["ot"] = opsum.tile([128, QG], FP, tag="ot", name=f"ot{hp}_{g}")
            for hh in (0, 1):
                nc.tensor.matmul(
                    st["ot"][64 * hh : 64 * (hh + 1), w0:],
                    lhsT=v_sb[:, hp, j, 64 * hh : 64 * (hh + 1)],
                    rhs=e[:, hh, w0:],
                    start=(j == 0),
                    stop=(j == njs - 1),
                )
            if j == njs - 1:
                finalize()

        def finalize():
            # denominator broadcast: dbc[p, q] = sum_k ES[k, head(p), q]
            dbc = fillp.tile([128, QG], FP, tag="fill", name=f"dbc{hp}_{g}")
            for hh in (0, 1):
                nc.tensor.matmul(
                    dbc[64 * hh : 64 * (hh + 1), :],
                    lhsT=ones_bf,
                    rhs=st["es"][:, hh, :],
                    start=True,
                    stop=True,
                )
            recip = fin.tile([128, QG], FP, tag="recip", name=f"rc{hp}_{g}")
            nc.vector.reciprocal_approx_fast(recip, dbc)
            o_sb = fin.tile([128, QG], FP, tag="o_sb", name=f"o{hp}_{g}")
            nc.vector.tensor_mul(o_sb, st["ot"], recip)
            nc.sync.dma_start(out[hp * 128 : (hp + 1) * 128, qsl], o_sb)

        return emit_qk, emit_av, njs

    pending = []
    for hp in range(HPC // 2):
        for g in range(nqg):
            gate(2 * (hp * ntc + g))      # q/k of chunks <= g ready
            emit_qk, emit_av, njs = run_group(hp, g)
            gated_v = False
            # pace: next group's qk+v stages should finish by group end
            deadline = 2 * (hp * ntc + g + 1) + 1
            for j in range(njs):
                e, w0 = emit_qk(j)
                pending.append((emit_av, j, e, w0))
                if not gated_v:
                    gate(2 * (hp * ntc + g) + 1)  # v of chunks <= g ready
                    gated_v = True
                if len(pending) > 5:
                    av, jj, e, ww = pending.pop(0)
                    av(jj, e, ww)
                if j % 2 == 0 or j == njs - 1:
                    feed_paced(deadline, (njs - j + 1) // 2)
    for av, jj, e, ww in pending:
        av(jj, e, ww)
    feed(10 ** 9)


def build_program(t=T_FULL):
    if t in _CACHE:
        return _CACHE[t]
    nc = bacc.Bacc("TRN2", target_bir_lowering=False, debug=False)
    with tile.TileContext(nc) as tc:
        with ExitStack() as ctx:
            _emit(ctx, tc, t)
    nc.compile()
    _CACHE[t] = nc
    return nc


def make_in_maps(x, Wq, Wk, Wv):
    """Host-side shard: returns the 8 per-core input maps."""
    x = np.asarray(x, dtype=np.float32)
    Wq = np.asarray(Wq, dtype=np.float32)
    Wk = np.asarray(Wk, dtype=np.float32)
    Wv = np.asarray(Wv, dtype=np.float32)
    scale = np.float32(D ** -0.5)
    bf = ml_dtypes.bfloat16
    xT = np.ascontiguousarray(x.transpose(0, 2, 1)).astype(bf)  # [B, C, T]
    in_maps = []
    for core in range(N_CORES):
        b, hg = divmod(core, N_CORES // B)
        sl = slice(hg * CPC, (hg + 1) * CPC)
        in_maps.append(
            {
                "xT": xT[b],
                "wqT": (np.ascontiguousarray(Wq[sl].T) * scale).astype(bf),
                "wkT": np.ascontiguousarray(Wk[sl].T).astype(bf),
                "wvT": np.ascontiguousarray(Wv[sl].T).astype(bf),
            }
        )
    return in_maps


LAST_RESULTS = None


def kernel(x, Wq, Wk, Wv, _trace=False):
    global LAST_RESULTS
    in_maps = make_in_maps(x, Wq, Wk, Wv)
    nc = build_program(T_FULL)
    res = bass_utils.run_bass_kernel_spmd(
        nc, in_maps, core_ids=list(range(N_CORES)), trace=_trace
    )
    LAST_RESULTS = res
    full = np.empty((B, T_FULL, C), np.float32)
    for core in range(N_CORES):
        b, hg = divmod(core, N_CORES // B)
        full[b, :, hg * CPC : (hg + 1) * CPC] = res.results[core]["out"].T
    return full
